# revision 4
# baseline (speedup 1.0000x reference)
"""CLSTMCell fused cell kernel for 8 Trainium2 NeuronCores — Gauss + Strassen.

Data-parallel over batch: each core takes a 512-row shard; weights replicated.

Complex multiply via Gauss's 3-mult trick (stacks k1/k2/k3n as in the fp16
baseline), then ONE level of Strassen on each stack's real matmul
  [512 batch x 2048 k] @ [2048 k x 256 n(gate pair block j)]
with splits: batch -> b1|b2 (256 each), k -> x-part|h-part (1024 each),
n -> gateA|gateB (128-col block j of each gate of the pair). 7 M-products
replace 8 quadrant products: PE work drops 12.5% below the fp16 roofline.

 M1=(A+D)(E+H) M2=(C+D)E M3=A(F-H) M4=D(G-E) M5=(A+B)H M6=(C-A)(E+F)
 M7=(B-D)(G+H);  C11=M1+M4-M5+M7 C21=M2+M4 C12=M3+M5 C22=M1-M2+M3+M6

Act combos (A+D etc) are built once during phases 0-1 on DVE (a1 combos
derived as a2combo+a3combo). Weight combos are prebuilt on the HOST and
streamed as 7 stationaries per phase (5.5 MB/phase lands right at the DMA
ridge; on-device combo builds would make DVE the bottleneck). Each M's three
Gauss-stack chains accumulate in PSUM [128,256] half-bank tiles; DVE drains
them into per-gate z accumulators [128, 4*256] in SBUF (quad order
C11|C21|C12|C22 = gA b1|gA b2|gB b1|gB b2) with ACT doing the first-touch
copies and all gate activations. M slot order M4,M5,M7,M1,M2,M3,M6 completes
gateA after slot 4 and gateB halves after slots 5/6 so combines overlap the
next slots' matmuls. Phases 0-1 instead run direct quad products from raw
E,G,H,F quarters (smaller startup DMA, no combo dependencies; the a1 B/C
quarters stream twice through the PE). Outputs stream back as fp16.
"""

import sys

sys.path.insert(0, "/opt/trn_rl_repo")

import numpy as np

import concourse.bacc as bacc
import concourse.mybir as mybir
import concourse.tile as tile
from concourse.bass_utils import run_bass_kernel_spmd

N_CORES = 8
B, D, U = 4096, 1024, 1024
BS = B // N_CORES          # 512 batch rows per core
HB = BS // 2               # 256 batch half
P = 128
KH = 8                     # k-blocks per k-half (x-part / h-part)
NJ = U // P                # 8 u-blocks
NPH = 2 * NJ               # 16 phases
PAIRS = ((0, 2), (1, 3))   # (i, c~), (f, o)
F32 = mybir.dt.float32
F16 = mybir.dt.float16
ADD = mybir.AluOpType.add
SUB = mybir.AluOpType.subtract
MULT = mybir.AluOpType.mult
MIN = mybir.AluOpType.min

# M slots: (name, moving, stationary, [(quad_colbase, sign)...])
# moving: ("raw", a2slot, a3slot) uses araw slots + a1 tile; ("cmb", c) uses acm
# stationary: ("raw", kindbase) uses wraw; ("cmb",) uses the slot's jit tile
# wraw dim1: E:0-2 G:3-5 H:6-8 F:9-11 (x3 stacks); araw slots:
# a2A0 a3A1 a2D2 a3D3 a2B4 a3B5 a2C6 a3C7
# acm combos c: T1(A+D)=0 T2(C+D)=1 T5(A+B)=2 T6(C-A)=3 T7(B-D)=4
# quads in accum [P, 4*HB]: C11@0 C21@256 C12@512 C22@768
WB = {"GE": 0, "H": 3, "GH": 6, "EH": 9, "E": 12, "FH": 15, "EF": 18}
SLOTS = (
    ("M4", ("rawD",), "GE", ((0, 1), (256, 1))),
    ("M5", ("cmb", 2), "H", ((0, -1), (512, 1))),
    ("M7", ("cmb", 4), "GH", ((0, 1),)),
    ("M1", ("cmb", 0), "EH", ((0, 1), (768, 1))),
    ("M2", ("cmb", 1), "E", ((256, 1), (768, -1))),
    ("M3", ("rawA",), "FH", ((512, 1), (768, 1))),
    ("M6", ("cmb", 3), "EF", ((768, 1),)),
)

_CACHE = {}


def _build():
    nc = bacc.Bacc("TRN2", target_bir_lowering=False, debug=False,
                   num_devices=N_CORES)
    Tanh = mybir.ActivationFunctionType.Tanh
    Relu = mybir.ActivationFunctionType.Relu

    aq_in = nc.dram_tensor("aq", [P, 8 * KH * HB], F16, kind="ExternalInput").ap()
    wq_in = nc.dram_tensor("wq", [P, NPH * 21 * KH * P], F16,
                           kind="ExternalInput").ap()
    w01_in = nc.dram_tensor("wq01", [P, 2 * 12 * KH * P], F16,
                            kind="ExternalInput").ap()
    cprev_in = nc.dram_tensor("c_prevT", [P, 2 * NJ * BS], F32,
                              kind="ExternalInput").ap()
    brT = nc.dram_tensor("brT", [P, 4 * NJ], F32, kind="ExternalInput").ap()
    biT = nc.dram_tensor("biT", [P, 4 * NJ], F32, kind="ExternalInput").ap()
    h_outT = nc.dram_tensor("h_outT", [2 * U, BS], F16, kind="ExternalOutput").ap()
    c_outT = nc.dram_tensor("c_outT", [2 * U, BS], F16, kind="ExternalOutput").ap()

    WPH = 21 * KH * P   # weight elems per phase per partition

    with tile.TileContext(nc) as tc:
        with (
            tc.tile_pool(name="acts", bufs=1) as acts_p,
            tc.tile_pool(name="wraw", bufs=2) as wraw_p,
            tc.tile_pool(name="accum", bufs=2) as acc_p,
            tc.tile_pool(name="cprev", bufs=4) as cp_p,
            tc.tile_pool(name="bias", bufs=4) as bias_p,
            tc.tile_pool(name="gat", bufs=5) as gat_p,
            tc.tile_pool(name="out", bufs=5) as out_p,
            tc.tile_pool(name="psum", bufs=8, space="PSUM") as psum_p,
        ):
            # --- PE p-state warmup on zeros while startup DMA streams
            warm = out_p.tile([P, BS], F16, tag="out", name="warm")
            nc.gpsimd.memset(warm[:], 0)
            wps = psum_p.tile([P, BS], F32, tag="ps", name="warm_ps")
            for _ in range(12):
                nc.tensor.matmul(wps[:], warm[:, :P], warm[:],
                                 start=True, stop=True)

            # --- startup DMAs ------------------------------------------------
            # acts quarters, critical order A, D, B, C. A/D persist; B/C
            # share the cprev ring (dead after combo builds)
            aslot = []
            for q in range(8):
                if q < 4:
                    t = acts_p.tile([P, KH, HB], F16, tag="arawAD", bufs=4,
                                    name=f"araw{q}")
                else:
                    t = cp_p.tile([P, KH, HB], F16, tag="cp", bufs=4,
                                  name=f"araw{q}")
                aslot.append(t)
                eng = nc.scalar if q < 4 else nc.gpsimd
                eng.dma_start(t[:], aq_in[:, q * KH * HB:
                                          (q + 1) * KH * HB])

            wtiles = {}

            def wdma(ph):
                wt = wraw_p.tile([P, 21, KH, P], F16, tag="wraw",
                                 name=f"wraw{ph}")
                wtiles[ph] = wt
                for c0, c1 in ((0, 12), (12, 21)):
                    nc.sync.dma_start(
                        wt[:, c0:c1, :, :],
                        wq_in[:, ph * WPH + c0 * KH * P:
                              ph * WPH + c1 * KH * P])
                return wt

            # phases 0/1 stream raw E,G,H,F (smaller startup DMA); their
            # combos are not needed: those phases run direct quad products
            W01 = 12 * KH * P

            def wdma01(ph):
                wt = wraw_p.tile([P, 12, KH, P], F16, tag="wraw",
                                 name=f"wraw01_{ph}")
                wtiles[ph] = wt
                for c0, c1 in ((0, 6), (6, 12)):
                    nc.sync.dma_start(
                        wt[:, c0:c1, :, :],
                        w01_in[:, ph * W01 + c0 * KH * P:
                               ph * W01 + c1 * KH * P])

            wdma01(0)
            wdma01(1)

            # biases (small, needed at phase0 combine)
            braw, bhs = [], []
            for nm, din in (("brT", brT), ("biT", biT)):
                t = bias_p.tile([P, 4 * NJ], F32, tag="bias", name=f"braw_{nm}")
                nc.gpsimd.dma_start(t[:], din[:, :])
                braw.append(t)
                t2 = bias_p.tile([P, 4 * NJ], F32, tag="bias", name=f"bhs_{nm}")
                nc.vector.tensor_scalar(t2[:], t[:], 0.2, 0.5, MULT, ADD)
                bhs.append(t2)

            # --- act combos (one-time) --------------------------------------
            # a1 raw A/D quarters + 5 combos x 3 stacks; a1 combos derived
            a1A = acts_p.tile([P, KH, HB], F16, tag="a1A", name="a1A")
            a1D = acts_p.tile([P, KH, HB], F16, tag="a1D", name="a1D")
            acmt = [acts_p.tile([P, KH, HB], F16, tag="acm", bufs=15,
                                name=f"acm{i}") for i in range(15)]
            QA = {1: 0, 2: 1}   # stack -> araw slot of quarter A (a2A, a3A)
            QD = {1: 2, 2: 3}
            QB = {1: 4, 2: 5}
            QC = {1: 6, 2: 7}
            nc.vector.tensor_tensor(a1A[:], aslot[0][:], aslot[1][:], ADD)
            nc.vector.tensor_tensor(a1D[:], aslot[2][:], aslot[3][:], ADD)
            # combos: (c, q0map, q1map, op); built during phase 1
            CDEFS = ((2, QA, QB, ADD), (4, QB, QD, SUB), (0, QA, QD, ADD),
                     (1, QC, QD, ADD), (3, QC, QA, SUB))

            def acm_build():
                for c, q0, q1, op in CDEFS:
                    for s in (1, 2):
                        nc.vector.tensor_tensor(acmt[c * 3 + s][:],
                                                aslot[q0[s]][:],
                                                aslot[q1[s]][:], op)
                    nc.vector.tensor_tensor(acmt[c * 3][:], acmt[c * 3 + 1][:],
                                            acmt[c * 3 + 2][:], ADD)

            def moving(mv, s, k):
                if mv[0] == "rawD":
                    return a1D[:, k, :] if s == 0 else aslot[QD[s]][:, k, :]
                if mv[0] == "rawA":
                    return a1A[:, k, :] if s == 0 else aslot[QA[s]][:, k, :]
                return acmt[mv[1] * 3 + s][:, k, :]

            # state carried across phases
            t2s = {}
            cpt = {}

            RB = {"E": 0, "G": 3, "H": 6, "F": 9}
            DSLOTS = (
                (0, ("A", "B"), ("E", "G"), None),
                (256, ("C", "D"), ("E", "G"), "gA"),
                (512, ("A", "B"), ("F", "H"), "gB0"),
                (768, ("C", "D"), ("F", "H"), "gB1"),
            )
            DQ = {"A": (a1A, 0, 1), "D": (a1D, 2, 3),
                  "B": (None, 4, 5), "C": (None, 6, 7)}

            def dmov(qq, s):
                t0, i1, i2 = DQ[qq]
                if s == 1:
                    return [aslot[i1]]
                if s == 2:
                    return [aslot[i2]]
                return [t0] if t0 is not None else [aslot[i1], aslot[i2]]

            for ph in range(NPH):
                j, pair = divmod(ph, 2)
                gA, gB = PAIRS[pair]
                wt = wtiles.pop(ph)
                if pair == 1:
                    t = cp_p.tile([P, 2, BS], F32, tag="cp", bufs=4,
                                  name=f"cp{j}")
                    nc.sync.dma_start(t[:], cprev_in[:, 2 * j * BS:
                                                     (2 * j + 2) * BS])
                    cpt[j] = t
                if ph + 2 < NPH:
                    wdma(ph + 2)
                if ph == 1:
                    acm_build()

                acc = [acc_p.tile([P, 4 * HB], F32, tag="acc",
                                  name=f"acc_{ph}_{z}") for z in range(2)]
                state = {}

                def combine_gA(ph=ph, j=j, pair=pair, gA=gA, acc=acc,
                               state=state):
                    for z in range(2):
                        g = gat_p.tile([P, BS], F16, tag="gat",
                                       name=f"ga{ph}{z}")
                        nc.scalar.activation(
                            g[:], acc[z][:, :BS], Relu,
                            bias=bhs[z][:, gA * NJ + j:gA * NJ + j + 1],
                            scale=0.2)
                        if pair == 0:
                            state.setdefault("igs", []).append(g)
                        else:
                            t1 = gat_p.tile([P, BS], F16, tag="gat",
                                            name=f"t1{ph}{z}")
                            nc.vector.scalar_tensor_tensor(
                                t1[:], g[:], 1.0, cpt[j][:, z], MIN, MULT)
                            cn = out_p.tile([P, BS], F16, tag="out",
                                            name=f"cn{ph}{z}")
                            nc.vector.tensor_tensor(cn[:], t1[:],
                                                    t2s[z][:], ADD)
                            rows0 = z * U + j * P
                            nc.sync.dma_start(c_outT[rows0:rows0 + P, :],
                                              cn[:])
                            tc2 = gat_p.tile([P, BS], F16, tag="gat",
                                             name=f"tc2{ph}{z}")
                            nc.scalar.activation(tc2[:], cn[:], Tanh)
                            t2s[("tc2", z)] = tc2

                def combine_gB(hf, ph=ph, j=j, pair=pair, gB=gB, acc=acc,
                               state=state):
                    asl = slice(BS + hf * HB, BS + (hf + 1) * HB)
                    gsl = slice(hf * HB, (hf + 1) * HB)
                    if pair == 0:
                        if hf == 0:
                            state["gbt"] = [
                                (gat_p.tile([P, BS], F16, tag="gat",
                                            name=f"tt{ph}{z}"),
                                 gat_p.tile([P, BS], F16, tag="gat",
                                            name=f"t2{ph}{z}"))
                                for z in range(2)]
                        for z in range(2):
                            tt, t2 = state["gbt"][z]
                            nc.scalar.activation(
                                tt[:, gsl], acc[z][:, asl], Tanh,
                                bias=braw[z][:, gB * NJ + j:gB * NJ + j + 1],
                                scale=1.0)
                            nc.vector.scalar_tensor_tensor(
                                t2[:, gsl], state["igs"][z][:, gsl], 1.0,
                                tt[:, gsl], MIN, MULT)
                            t2s[z] = t2
                    else:
                        if hf == 0:
                            state["gbo"] = [
                                (gat_p.tile([P, BS], F16, tag="gat",
                                            name=f"og{ph}{z}"),
                                 out_p.tile([P, BS], F16, tag="out",
                                            name=f"hn{ph}{z}"))
                                for z in range(2)]
                        for z in range(2):
                            og, hn = state["gbo"][z]
                            nc.scalar.activation(
                                og[:, gsl], acc[z][:, asl], Relu,
                                bias=bhs[z][:, gB * NJ + j:gB * NJ + j + 1],
                                scale=0.2)
                            nc.vector.scalar_tensor_tensor(
                                hn[:, gsl], og[:, gsl], 1.0,
                                t2s[("tc2", z)][:, gsl], MIN, MULT)
                            rows0 = z * U + j * P
                            nc.scalar.dma_start(
                                h_outT[rows0:rows0 + P, gsl], hn[:, gsl])

                if ph < 2:
                    # direct quad products from raw E,G,H,F: no combo deps,
                    # smaller startup DMA; a1 B/C quarters stream twice on PE
                    for cb, qs, sts, trig in DSLOTS:
                        pa = psum_p.tile([P, BS], F32, tag="ps",
                                         name=f"dpa{ph}{cb}")
                        pb = psum_p.tile([P, BS], F32, tag="ps",
                                         name=f"dpb{ph}{cb}")
                        ps = (pa[:, :HB], pa[:, HB:], pb[:, :HB])
                        for s in (1, 2, 0):
                            seq = []
                            for qq, kind in zip(qs, sts):
                                for m in dmov(qq, s):
                                    seq.append((kind, m))
                            n = len(seq) * KH
                            t = 0
                            for kind, m in seq:
                                for k in range(KH):
                                    nc.tensor.matmul(
                                        ps[s], wt[:, RB[kind] + s, k, :],
                                        m[:, k, :],
                                        start=(t == 0), stop=(t == n - 1))
                                    t += 1
                        for z, sx in ((0, 2), (1, 1)):
                            a = acc[z][:, cb:cb + HB]
                            nc.scalar.copy(a, ps[0])
                            nc.vector.tensor_tensor(a, a, ps[sx], ADD)
                        if trig == "gA":
                            combine_gA()
                        elif trig == "gB0":
                            combine_gB(0)
                        elif trig == "gB1":
                            combine_gB(1)
                    continue

                inited = set()
                for si, (mn, mv, st, uses) in enumerate(SLOTS):
                    pa = psum_p.tile([P, BS], F32, tag="ps", name=f"pa{ph}{si}")
                    pb = psum_p.tile([P, BS], F32, tag="ps", name=f"pb{ph}{si}")
                    ps = (pa[:, :HB], pa[:, HB:], pb[:, :HB])  # s0, s1, s2
                    for s in (1, 2, 0):
                        stat = wt[:, WB[st] + s]
                        for k in range(KH):
                            nc.tensor.matmul(ps[s], stat[:, k, :],
                                             moving(mv, s, k),
                                             start=(k == 0), stop=(k == KH - 1))
                    # drains: zr uses s0+s2, zi uses s0+s1; init via ACT copy
                    for cb, sgn in uses:
                        for z, sx in ((0, 2), (1, 1)):
                            a = acc[z][:, cb:cb + HB]
                            if (cb, z) not in inited:
                                inited.add((cb, z))
                                assert sgn == 1
                                nc.scalar.copy(a, ps[0])
                                nc.vector.tensor_tensor(a, a, ps[sx], ADD)
                            else:
                                op = ADD if sgn == 1 else SUB
                                nc.vector.tensor_tensor(a, a, ps[0], op)
                                nc.vector.tensor_tensor(a, a, ps[sx], op)

                    if si == 4:
                        combine_gA()
                    elif si in (5, 6):
                        combine_gB(si - 5)

    nc.compile()
    return nc


def _in_maps(inputs, h_tm1, c_tm1, wr, wi, wrr, wir, br, bi):
    brT = np.ascontiguousarray(br.reshape(4 * NJ, P).T)
    biT = np.ascontiguousarray(bi.reshape(4 * NJ, P).T)

    # Gauss weight stacks
    W1 = np.concatenate([wr, wrr], 0)
    W2 = np.concatenate([-(wi + wr), -(wir + wrr)], 0)
    W3 = np.concatenate([wi - wr, wir - wrr], 0)
    # 7 stationaries per phase/stack: raw E,H + 5 host-built combos
    WQ = np.empty((NPH, 21, P, KH, P), np.float16)
    for s, Ws in enumerate((W1, W2, W3)):
        v = Ws.reshape(2, KH, P, 4, NJ, P)
        for ph in range(NPH):
            j, pair = divmod(ph, 2)
            gA, gB = PAIRS[pair]
            E = v[0, :, :, gA, j, :]
            G = v[1, :, :, gA, j, :]
            H = v[1, :, :, gB, j, :]
            F = v[0, :, :, gB, j, :]
            for kind, arr in (("GE", G - E), ("H", H), ("GH", G + H),
                              ("EH", E + H), ("E", E), ("FH", F - H),
                              ("EF", E + F)):
                WQ[ph, WB[kind] + s] = arr.astype(np.float16).transpose(1, 0, 2)
    wq = np.ascontiguousarray(WQ.transpose(2, 0, 1, 3, 4).reshape(P, -1))
    RB = {"E": 0, "G": 3, "H": 6, "F": 9}
    W01 = np.empty((2, 12, P, KH, P), np.float16)
    for s, Ws in enumerate((W1, W2, W3)):
        v = Ws.reshape(2, KH, P, 4, NJ, P)
        for ph in range(2):
            gA, gB = PAIRS[ph]
            for kind, arr in (("E", v[0, :, :, gA, 0, :]),
                              ("G", v[1, :, :, gA, 0, :]),
                              ("H", v[1, :, :, gB, 0, :]),
                              ("F", v[0, :, :, gB, 0, :])):
                W01[ph, RB[kind] + s] = arr.astype(np.float16).transpose(1, 0, 2)
    wq01 = np.ascontiguousarray(W01.transpose(2, 0, 1, 3, 4).reshape(P, -1))

    maps = []
    for c in range(N_CORES):
        rows = slice(c * BS, (c + 1) * BS)
        xr, xi_ = inputs[rows, :D], inputs[rows, D:]
        hr, hi = h_tm1[rows, :U], h_tm1[rows, U:]
        a2 = np.empty((D + U, BS), np.float16)
        a2[:D] = xr.T
        a2[D:] = hr.T
        a3 = np.empty((D + U, BS), np.float16)
        a3[:D] = xi_.T
        a3[D:] = hi.T
        # quarter slots: a2A a3A a2D a3D a2B a3B a2C a3C
        # A=(k1,b1) B=(k2,b1) C=(k1,b2) D=(k2,b2)
        aq = np.empty((8, P, KH, HB), np.float16)
        for i, (src, kh, bh) in enumerate((
                (a2, 0, 0), (a3, 0, 0), (a2, 1, 1), (a3, 1, 1),
                (a2, 1, 0), (a3, 1, 0), (a2, 0, 1), (a3, 0, 1))):
            q = src[kh * 1024:(kh + 1) * 1024, bh * HB:(bh + 1) * HB]
            aq[i] = q.reshape(KH, P, HB).transpose(1, 0, 2)
        aqm = np.ascontiguousarray(aq.transpose(1, 0, 2, 3).reshape(P, -1))

        cpv = c_tm1[rows].T.reshape(2, NJ, P, BS).transpose(2, 1, 0, 3)
        cpv = np.ascontiguousarray(cpv.reshape(P, 2 * NJ * BS), np.float32)
        maps.append({"aq": aqm, "wq": wq, "wq01": wq01, "c_prevT": cpv,
                     "brT": brT, "biT": biT})
    return maps


def kernel(inputs, h_tm1, c_tm1, real_kernel, imaginary_kernel,
           real_recurrent_kernel, imaginary_recurrent_kernel,
           real_bias, imaginary_bias):
    if "nc" not in _CACHE:
        _CACHE["nc"] = _build()
    nc = _CACHE["nc"]

    maps = _in_maps(
        np.ascontiguousarray(inputs, dtype=np.float32),
        np.ascontiguousarray(h_tm1, dtype=np.float32),
        np.ascontiguousarray(c_tm1, dtype=np.float32),
        np.ascontiguousarray(real_kernel, dtype=np.float32),
        np.ascontiguousarray(imaginary_kernel, dtype=np.float32),
        np.ascontiguousarray(real_recurrent_kernel, dtype=np.float32),
        np.ascontiguousarray(imaginary_recurrent_kernel, dtype=np.float32),
        np.ascontiguousarray(real_bias, dtype=np.float32),
        np.ascontiguousarray(imaginary_bias, dtype=np.float32),
    )
    res = run_bass_kernel_spmd(nc, maps, list(range(N_CORES)))
    h = np.concatenate(
        [res.results[c]["h_outT"].T.astype(np.float32)
         for c in range(N_CORES)], axis=0)
    c = np.concatenate(
        [res.results[c]["c_outT"].T.astype(np.float32)
         for c in range(N_CORES)], axis=0)
    return np.ascontiguousarray(h), np.ascontiguousarray(c)


# revision 5
# speedup vs baseline: 1.0227x; 1.0227x over previous
"""CLSTMCell fused cell kernel for 8 Trainium2 NeuronCores — Gauss + Strassen.

Data-parallel over batch: each core takes a 512-row shard; weights replicated.

Complex multiply via Gauss's 3-mult trick (stacks k1/k2/k3n as in the fp16
baseline), then ONE level of Strassen on each stack's real matmul
  [512 batch x 2048 k] @ [2048 k x 256 n(gate pair block j)]
with splits: batch -> b1|b2 (256 each), k -> x-part|h-part (1024 each),
n -> gateA|gateB (128-col block j of each gate of the pair). 7 M-products
replace 8 quadrant products: PE work drops 12.5% below the fp16 roofline.

 M1=(A+D)(E+H) M2=(C+D)E M3=A(F-H) M4=D(G-E) M5=(A+B)H M6=(C-A)(E+F)
 M7=(B-D)(G+H);  C11=M1+M4-M5+M7 C21=M2+M4 C12=M3+M5 C22=M1-M2+M3+M6

Act combos (A+D etc) are built once during phases 0-1 on DVE (a1 combos
derived as a2combo+a3combo). Weight combos are prebuilt on the HOST and
streamed as 7 stationaries per phase (5.5 MB/phase lands right at the DMA
ridge; on-device combo builds would make DVE the bottleneck). Each M's three
Gauss-stack chains accumulate in PSUM [128,256] half-bank tiles; DVE drains
them into per-gate z accumulators [128, 4*256] in SBUF (quad order
C11|C21|C12|C22 = gA b1|gA b2|gB b1|gB b2) with ACT doing the first-touch
copies and all gate activations. M slot order M4,M5,M7,M1,M2,M3,M6 completes
gateA after slot 4 and gateB halves after slots 5/6 so combines overlap the
next slots' matmuls. Phases 0-1 instead run direct quad products from raw
E,G,H,F quarters (smaller startup DMA, no combo dependencies; the a1 B/C
quarters stream twice through the PE). Outputs stream back as fp16.
"""

import sys

sys.path.insert(0, "/opt/trn_rl_repo")

import numpy as np

import concourse.bacc as bacc
import concourse.mybir as mybir
import concourse.tile as tile
from concourse.bass_utils import run_bass_kernel_spmd

N_CORES = 8
B, D, U = 4096, 1024, 1024
BS = B // N_CORES          # 512 batch rows per core
HB = BS // 2               # 256 batch half
P = 128
KH = 8                     # k-blocks per k-half (x-part / h-part)
NJ = U // P                # 8 u-blocks
NPH = 2 * NJ               # 16 phases
PAIRS = ((0, 2), (1, 3))   # (i, c~), (f, o)
F32 = mybir.dt.float32
F16 = mybir.dt.float16
ADD = mybir.AluOpType.add
SUB = mybir.AluOpType.subtract
MULT = mybir.AluOpType.mult
MIN = mybir.AluOpType.min

# M slots: (name, moving, stationary, [(quad_colbase, sign)...])
# moving: ("raw", a2slot, a3slot) uses araw slots + a1 tile; ("cmb", c) uses acm
# stationary: ("raw", kindbase) uses wraw; ("cmb",) uses the slot's jit tile
# wraw dim1: E:0-2 G:3-5 H:6-8 F:9-11 (x3 stacks); araw slots:
# a2A0 a3A1 a2D2 a3D3 a2B4 a3B5 a2C6 a3C7
# acm combos c: T1(A+D)=0 T2(C+D)=1 T5(A+B)=2 T6(C-A)=3 T7(B-D)=4
# quads in accum [P, 4*HB]: C11@0 C21@256 C12@512 C22@768
WB = {"GE": 0, "H": 3, "GH": 6, "EH": 9, "E": 12, "FH": 15, "EF": 18}
SLOTS = (
    ("M4", ("rawD",), "GE", ((0, 1), (256, 1))),
    ("M5", ("cmb", 2), "H", ((0, -1), (512, 1))),
    ("M7", ("cmb", 4), "GH", ((0, 1),)),
    ("M1", ("cmb", 0), "EH", ((0, 1), (768, 1))),
    ("M2", ("cmb", 1), "E", ((256, 1), (768, -1))),
    ("M3", ("rawA",), "FH", ((512, 1), (768, 1))),
    ("M6", ("cmb", 3), "EF", ((768, 1),)),
)

_CACHE = {}


def _build():
    nc = bacc.Bacc("TRN2", target_bir_lowering=False, debug=False,
                   num_devices=N_CORES)
    Tanh = mybir.ActivationFunctionType.Tanh
    Relu = mybir.ActivationFunctionType.Relu

    aq_in = nc.dram_tensor("aq", [P, 8 * KH * HB], F16, kind="ExternalInput").ap()
    wq_in = nc.dram_tensor("wq", [P, NPH * 21 * KH * P], F16,
                           kind="ExternalInput").ap()
    w01_in = nc.dram_tensor("wq01", [P, 2 * 12 * KH * P], F16,
                            kind="ExternalInput").ap()
    cprev_in = nc.dram_tensor("c_prevT", [P, 2 * NJ * BS], F32,
                              kind="ExternalInput").ap()
    brT = nc.dram_tensor("brT", [P, 4 * NJ], F32, kind="ExternalInput").ap()
    biT = nc.dram_tensor("biT", [P, 4 * NJ], F32, kind="ExternalInput").ap()
    h_outT = nc.dram_tensor("h_outT", [2 * U, BS], F16, kind="ExternalOutput").ap()
    c_outT = nc.dram_tensor("c_outT", [2 * U, BS], F16, kind="ExternalOutput").ap()

    WPH = 21 * KH * P   # weight elems per phase per partition

    with tile.TileContext(nc) as tc:
        with (
            tc.tile_pool(name="acts", bufs=1) as acts_p,
            tc.tile_pool(name="wraw", bufs=2) as wraw_p,
            tc.tile_pool(name="accum", bufs=2) as acc_p,
            tc.tile_pool(name="cprev", bufs=4) as cp_p,
            tc.tile_pool(name="bias", bufs=4) as bias_p,
            tc.tile_pool(name="gat", bufs=5) as gat_p,
            tc.tile_pool(name="out", bufs=5) as out_p,
            tc.tile_pool(name="psum", bufs=8, space="PSUM") as psum_p,
        ):
            # --- PE p-state warmup on zeros while startup DMA streams
            warm = out_p.tile([P, BS], F16, tag="out", name="warm")
            nc.gpsimd.memset(warm[:], 0)
            wps = psum_p.tile([P, BS], F32, tag="ps", name="warm_ps")
            for _ in range(12):
                nc.tensor.matmul(wps[:], warm[:, :P], warm[:],
                                 start=True, stop=True)

            # --- startup DMAs ------------------------------------------------
            # acts quarters, critical order A, D, B, C. A/D persist; B/C
            # share the cprev ring (dead after combo builds)
            aslot = []
            for q in range(8):
                if q < 4:
                    t = acts_p.tile([P, KH, HB], F16, tag="arawAD", bufs=4,
                                    name=f"araw{q}")
                else:
                    t = cp_p.tile([P, KH, HB], F16, tag="cp", bufs=4,
                                  name=f"araw{q}")
                aslot.append(t)
                eng = nc.scalar if q < 4 else nc.gpsimd
                eng.dma_start(t[:], aq_in[:, q * KH * HB:
                                          (q + 1) * KH * HB])

            wtiles = {}

            def wdma(ph):
                wt = wraw_p.tile([P, 21, KH, P], F16, tag="wraw",
                                 name=f"wraw{ph}")
                wtiles[ph] = wt
                for c0, c1 in ((0, 12), (12, 21)):
                    nc.sync.dma_start(
                        wt[:, c0:c1, :, :],
                        wq_in[:, ph * WPH + c0 * KH * P:
                              ph * WPH + c1 * KH * P])
                return wt

            # phases 0/1 stream raw E,G,H,F (smaller startup DMA); their
            # combos are not needed: those phases run direct quad products
            W01 = 12 * KH * P

            def wdma01(ph):
                wt = wraw_p.tile([P, 12, KH, P], F16, tag="wraw",
                                 name=f"wraw01_{ph}")
                wtiles[ph] = wt
                for c0, c1 in ((0, 6), (6, 12)):
                    nc.sync.dma_start(
                        wt[:, c0:c1, :, :],
                        w01_in[:, ph * W01 + c0 * KH * P:
                               ph * W01 + c1 * KH * P])

            wdma01(0)
            wdma01(1)

            # biases (small, needed at phase0 combine)
            braw, bhs = [], []
            for nm, din in (("brT", brT), ("biT", biT)):
                t = bias_p.tile([P, 4 * NJ], F32, tag="bias", name=f"braw_{nm}")
                nc.gpsimd.dma_start(t[:], din[:, :])
                braw.append(t)
                t2 = bias_p.tile([P, 4 * NJ], F32, tag="bias", name=f"bhs_{nm}")
                nc.vector.tensor_scalar(t2[:], t[:], 0.2, 0.5, MULT, ADD)
                bhs.append(t2)

            # --- act combos (one-time) --------------------------------------
            # a1 raw A/D quarters + 5 combos x 3 stacks; a1 combos derived
            a1A = acts_p.tile([P, KH, HB], F16, tag="a1A", name="a1A")
            a1D = acts_p.tile([P, KH, HB], F16, tag="a1D", name="a1D")
            acmt = [acts_p.tile([P, KH, HB], F16, tag="acm", bufs=15,
                                name=f"acm{i}") for i in range(15)]
            QA = {1: 0, 2: 1}   # stack -> araw slot of quarter A (a2A, a3A)
            QD = {1: 2, 2: 3}
            QB = {1: 4, 2: 5}
            QC = {1: 6, 2: 7}
            nc.vector.tensor_tensor(a1A[:], aslot[0][:], aslot[1][:], ADD)
            nc.vector.tensor_tensor(a1D[:], aslot[2][:], aslot[3][:], ADD)
            # a1 B/C quarters for the direct phases live in borrowed acm-ring
            # buffers (acmt[0],acmt[1]); the T1 combos overwrite them during
            # phase 1, after the last direct-phase read
            a1B = acts_p.tile([P, KH, HB], F16, tag="acm", bufs=15,
                              name="a1B")
            a1C = acts_p.tile([P, KH, HB], F16, tag="acm", bufs=15,
                              name="a1C")
            nc.vector.tensor_tensor(a1B[:], aslot[4][:], aslot[5][:], ADD)
            nc.vector.tensor_tensor(a1C[:], aslot[6][:], aslot[7][:], ADD)
            # combos: (c, q0map, q1map, op); built during phase 1
            CDEFS = ((2, QA, QB, ADD), (4, QB, QD, SUB), (1, QC, QD, ADD),
                     (3, QC, QA, SUB), (0, QA, QD, ADD))

            def acm_build():
                for c, q0, q1, op in CDEFS:
                    for s in (1, 2):
                        nc.vector.tensor_tensor(acmt[c * 3 + s][:],
                                                aslot[q0[s]][:],
                                                aslot[q1[s]][:], op)
                    nc.vector.tensor_tensor(acmt[c * 3][:], acmt[c * 3 + 1][:],
                                            acmt[c * 3 + 2][:], ADD)

            def moving(mv, s, k):
                if mv[0] == "rawD":
                    return a1D[:, k, :] if s == 0 else aslot[QD[s]][:, k, :]
                if mv[0] == "rawA":
                    return a1A[:, k, :] if s == 0 else aslot[QA[s]][:, k, :]
                return acmt[mv[1] * 3 + s][:, k, :]

            # state carried across phases
            t2s = {}
            cpt = {}

            RB = {"E": 0, "G": 3, "H": 6, "F": 9}
            DSLOTS = (
                (0, ("A", "B"), ("E", "G"), None),
                (256, ("C", "D"), ("E", "G"), "gA"),
                (512, ("A", "B"), ("F", "H"), "gB0"),
                (768, ("C", "D"), ("F", "H"), "gB1"),
            )
            DQ = {"A": (a1A, 0, 1), "D": (a1D, 2, 3),
                  "B": (a1B, 4, 5), "C": (a1C, 6, 7)}

            def dmov(qq, s):
                t0, i1, i2 = DQ[qq]
                if s == 1:
                    return [aslot[i1]]
                if s == 2:
                    return [aslot[i2]]
                return [t0] if t0 is not None else [aslot[i1], aslot[i2]]

            for ph in range(NPH):
                j, pair = divmod(ph, 2)
                gA, gB = PAIRS[pair]
                wt = wtiles.pop(ph)
                if pair == 1:
                    t = cp_p.tile([P, 2, BS], F32, tag="cp", bufs=4,
                                  name=f"cp{j}")
                    nc.sync.dma_start(t[:], cprev_in[:, 2 * j * BS:
                                                     (2 * j + 2) * BS])
                    cpt[j] = t
                if ph + 2 < NPH:
                    wdma(ph + 2)
                if ph == 1:
                    acm_build()

                acc = [acc_p.tile([P, 4 * HB], F32, tag="acc",
                                  name=f"acc_{ph}_{z}") for z in range(2)]
                state = {}

                def combine_gA(ph=ph, j=j, pair=pair, gA=gA, acc=acc,
                               state=state):
                    for z in range(2):
                        g = gat_p.tile([P, BS], F16, tag="gat",
                                       name=f"ga{ph}{z}")
                        nc.scalar.activation(
                            g[:], acc[z][:, :BS], Relu,
                            bias=bhs[z][:, gA * NJ + j:gA * NJ + j + 1],
                            scale=0.2)
                        if pair == 0:
                            state.setdefault("igs", []).append(g)
                        else:
                            t1 = gat_p.tile([P, BS], F16, tag="gat",
                                            name=f"t1{ph}{z}")
                            nc.vector.scalar_tensor_tensor(
                                t1[:], g[:], 1.0, cpt[j][:, z], MIN, MULT)
                            cn = out_p.tile([P, BS], F16, tag="out",
                                            name=f"cn{ph}{z}")
                            nc.vector.tensor_tensor(cn[:], t1[:],
                                                    t2s[z][:], ADD)
                            rows0 = z * U + j * P
                            nc.sync.dma_start(c_outT[rows0:rows0 + P, :],
                                              cn[:])
                            tc2 = gat_p.tile([P, BS], F16, tag="gat",
                                             name=f"tc2{ph}{z}")
                            nc.scalar.activation(tc2[:], cn[:], Tanh)
                            t2s[("tc2", z)] = tc2

                def combine_gB(hf, ph=ph, j=j, pair=pair, gB=gB, acc=acc,
                               state=state):
                    asl = slice(BS + hf * HB, BS + (hf + 1) * HB)
                    gsl = slice(hf * HB, (hf + 1) * HB)
                    if pair == 0:
                        if hf == 0:
                            state["gbt"] = [
                                (gat_p.tile([P, BS], F16, tag="gat",
                                            name=f"tt{ph}{z}"),
                                 gat_p.tile([P, BS], F16, tag="gat",
                                            name=f"t2{ph}{z}"))
                                for z in range(2)]
                        for z in range(2):
                            tt, t2 = state["gbt"][z]
                            nc.scalar.activation(
                                tt[:, gsl], acc[z][:, asl], Tanh,
                                bias=braw[z][:, gB * NJ + j:gB * NJ + j + 1],
                                scale=1.0)
                            nc.vector.scalar_tensor_tensor(
                                t2[:, gsl], state["igs"][z][:, gsl], 1.0,
                                tt[:, gsl], MIN, MULT)
                            t2s[z] = t2
                    else:
                        if hf == 0:
                            state["gbo"] = [
                                (gat_p.tile([P, BS], F16, tag="gat",
                                            name=f"og{ph}{z}"),
                                 out_p.tile([P, BS], F16, tag="out",
                                            name=f"hn{ph}{z}"))
                                for z in range(2)]
                        for z in range(2):
                            og, hn = state["gbo"][z]
                            nc.scalar.activation(
                                og[:, gsl], acc[z][:, asl], Relu,
                                bias=bhs[z][:, gB * NJ + j:gB * NJ + j + 1],
                                scale=0.2)
                            nc.vector.scalar_tensor_tensor(
                                hn[:, gsl], og[:, gsl], 1.0,
                                t2s[("tc2", z)][:, gsl], MIN, MULT)
                            rows0 = z * U + j * P
                            nc.scalar.dma_start(
                                h_outT[rows0:rows0 + P, gsl], hn[:, gsl])

                if ph < 2:
                    # direct quad products from raw E,G,H,F: no combo deps,
                    # smaller startup DMA; a1 B/C quarters stream twice on PE
                    for cb, qs, sts, trig in DSLOTS:
                        pa = psum_p.tile([P, BS], F32, tag="ps",
                                         name=f"dpa{ph}{cb}")
                        pb = psum_p.tile([P, BS], F32, tag="ps",
                                         name=f"dpb{ph}{cb}")
                        ps = (pa[:, :HB], pa[:, HB:], pb[:, :HB])
                        for s in (1, 2, 0):
                            seq = []
                            for qq, kind in zip(qs, sts):
                                for m in dmov(qq, s):
                                    seq.append((kind, m))
                            n = len(seq) * KH
                            t = 0
                            for kind, m in seq:
                                for k in range(KH):
                                    nc.tensor.matmul(
                                        ps[s], wt[:, RB[kind] + s, k, :],
                                        m[:, k, :],
                                        start=(t == 0), stop=(t == n - 1))
                                    t += 1
                        for z, sx in ((0, 2), (1, 1)):
                            a = acc[z][:, cb:cb + HB]
                            nc.scalar.copy(a, ps[0])
                            nc.vector.tensor_tensor(a, a, ps[sx], ADD)
                        if trig == "gA":
                            combine_gA()
                        elif trig == "gB0":
                            combine_gB(0)
                        elif trig == "gB1":
                            combine_gB(1)
                    continue

                inited = set()
                for si, (mn, mv, st, uses) in enumerate(SLOTS):
                    pa = psum_p.tile([P, BS], F32, tag="ps", name=f"pa{ph}{si}")
                    pb = psum_p.tile([P, BS], F32, tag="ps", name=f"pb{ph}{si}")
                    ps = (pa[:, :HB], pa[:, HB:], pb[:, :HB])  # s0, s1, s2
                    for s in (1, 2, 0):
                        stat = wt[:, WB[st] + s]
                        for k in range(KH):
                            nc.tensor.matmul(ps[s], stat[:, k, :],
                                             moving(mv, s, k),
                                             start=(k == 0), stop=(k == KH - 1))
                    # drains: zr uses s0+s2, zi uses s0+s1; init via ACT copy
                    for cb, sgn in uses:
                        for z, sx in ((0, 2), (1, 1)):
                            a = acc[z][:, cb:cb + HB]
                            if (cb, z) not in inited:
                                inited.add((cb, z))
                                assert sgn == 1
                                nc.scalar.copy(a, ps[0])
                                nc.vector.tensor_tensor(a, a, ps[sx], ADD)
                            else:
                                op = ADD if sgn == 1 else SUB
                                nc.vector.tensor_tensor(a, a, ps[0], op)
                                nc.vector.tensor_tensor(a, a, ps[sx], op)

                    if si == 4:
                        combine_gA()
                    elif si in (5, 6):
                        combine_gB(si - 5)

    nc.compile()
    return nc


def _in_maps(inputs, h_tm1, c_tm1, wr, wi, wrr, wir, br, bi):
    brT = np.ascontiguousarray(br.reshape(4 * NJ, P).T)
    biT = np.ascontiguousarray(bi.reshape(4 * NJ, P).T)

    # Gauss weight stacks
    W1 = np.concatenate([wr, wrr], 0)
    W2 = np.concatenate([-(wi + wr), -(wir + wrr)], 0)
    W3 = np.concatenate([wi - wr, wir - wrr], 0)
    # 7 stationaries per phase/stack: raw E,H + 5 host-built combos
    WQ = np.empty((NPH, 21, P, KH, P), np.float16)
    for s, Ws in enumerate((W1, W2, W3)):
        v = Ws.reshape(2, KH, P, 4, NJ, P)
        for ph in range(NPH):
            j, pair = divmod(ph, 2)
            gA, gB = PAIRS[pair]
            E = v[0, :, :, gA, j, :]
            G = v[1, :, :, gA, j, :]
            H = v[1, :, :, gB, j, :]
            F = v[0, :, :, gB, j, :]
            for kind, arr in (("GE", G - E), ("H", H), ("GH", G + H),
                              ("EH", E + H), ("E", E), ("FH", F - H),
                              ("EF", E + F)):
                WQ[ph, WB[kind] + s] = arr.astype(np.float16).transpose(1, 0, 2)
    wq = np.ascontiguousarray(WQ.transpose(2, 0, 1, 3, 4).reshape(P, -1))
    RB = {"E": 0, "G": 3, "H": 6, "F": 9}
    W01 = np.empty((2, 12, P, KH, P), np.float16)
    for s, Ws in enumerate((W1, W2, W3)):
        v = Ws.reshape(2, KH, P, 4, NJ, P)
        for ph in range(2):
            gA, gB = PAIRS[ph]
            for kind, arr in (("E", v[0, :, :, gA, 0, :]),
                              ("G", v[1, :, :, gA, 0, :]),
                              ("H", v[1, :, :, gB, 0, :]),
                              ("F", v[0, :, :, gB, 0, :])):
                W01[ph, RB[kind] + s] = arr.astype(np.float16).transpose(1, 0, 2)
    wq01 = np.ascontiguousarray(W01.transpose(2, 0, 1, 3, 4).reshape(P, -1))

    maps = []
    for c in range(N_CORES):
        rows = slice(c * BS, (c + 1) * BS)
        xr, xi_ = inputs[rows, :D], inputs[rows, D:]
        hr, hi = h_tm1[rows, :U], h_tm1[rows, U:]
        a2 = np.empty((D + U, BS), np.float16)
        a2[:D] = xr.T
        a2[D:] = hr.T
        a3 = np.empty((D + U, BS), np.float16)
        a3[:D] = xi_.T
        a3[D:] = hi.T
        # quarter slots: a2A a3A a2D a3D a2B a3B a2C a3C
        # A=(k1,b1) B=(k2,b1) C=(k1,b2) D=(k2,b2)
        aq = np.empty((8, P, KH, HB), np.float16)
        for i, (src, kh, bh) in enumerate((
                (a2, 0, 0), (a3, 0, 0), (a2, 1, 1), (a3, 1, 1),
                (a2, 1, 0), (a3, 1, 0), (a2, 0, 1), (a3, 0, 1))):
            q = src[kh * 1024:(kh + 1) * 1024, bh * HB:(bh + 1) * HB]
            aq[i] = q.reshape(KH, P, HB).transpose(1, 0, 2)
        aqm = np.ascontiguousarray(aq.transpose(1, 0, 2, 3).reshape(P, -1))

        cpv = c_tm1[rows].T.reshape(2, NJ, P, BS).transpose(2, 1, 0, 3)
        cpv = np.ascontiguousarray(cpv.reshape(P, 2 * NJ * BS), np.float32)
        maps.append({"aq": aqm, "wq": wq, "wq01": wq01, "c_prevT": cpv,
                     "brT": brT, "biT": biT})
    return maps


def kernel(inputs, h_tm1, c_tm1, real_kernel, imaginary_kernel,
           real_recurrent_kernel, imaginary_recurrent_kernel,
           real_bias, imaginary_bias):
    if "nc" not in _CACHE:
        _CACHE["nc"] = _build()
    nc = _CACHE["nc"]

    maps = _in_maps(
        np.ascontiguousarray(inputs, dtype=np.float32),
        np.ascontiguousarray(h_tm1, dtype=np.float32),
        np.ascontiguousarray(c_tm1, dtype=np.float32),
        np.ascontiguousarray(real_kernel, dtype=np.float32),
        np.ascontiguousarray(imaginary_kernel, dtype=np.float32),
        np.ascontiguousarray(real_recurrent_kernel, dtype=np.float32),
        np.ascontiguousarray(imaginary_recurrent_kernel, dtype=np.float32),
        np.ascontiguousarray(real_bias, dtype=np.float32),
        np.ascontiguousarray(imaginary_bias, dtype=np.float32),
    )
    res = run_bass_kernel_spmd(nc, maps, list(range(N_CORES)))
    h = np.concatenate(
        [res.results[c]["h_outT"].T.astype(np.float32)
         for c in range(N_CORES)], axis=0)
    c = np.concatenate(
        [res.results[c]["c_outT"].T.astype(np.float32)
         for c in range(N_CORES)], axis=0)
    return np.ascontiguousarray(h), np.ascontiguousarray(c)


# revision 6
# speedup vs baseline: 1.0277x; 1.0050x over previous
"""CLSTMCell fused cell kernel for 8 Trainium2 NeuronCores — Gauss + Strassen.

Data-parallel over batch: each core takes a 512-row shard; weights replicated.

Complex multiply via Gauss's 3-mult trick (stacks k1/k2/k3n as in the fp16
baseline), then ONE level of Strassen on each stack's real matmul
  [512 batch x 2048 k] @ [2048 k x 256 n(gate pair block j)]
with splits: batch -> b1|b2 (256 each), k -> x-part|h-part (1024 each),
n -> gateA|gateB (128-col block j of each gate of the pair). 7 M-products
replace 8 quadrant products: PE work drops 12.5% below the fp16 roofline.

 M1=(A+D)(E+H) M2=(C+D)E M3=A(F-H) M4=D(G-E) M5=(A+B)H M6=(C-A)(E+F)
 M7=(B-D)(G+H);  C11=M1+M4-M5+M7 C21=M2+M4 C12=M3+M5 C22=M1-M2+M3+M6

Act combos (A+D etc) are built once during phases 0-1 on DVE (a1 combos
derived as a2combo+a3combo). Weight combos are prebuilt on the HOST and
streamed as 7 stationaries per phase (5.5 MB/phase lands right at the DMA
ridge; on-device combo builds would make DVE the bottleneck). Each M's three
Gauss-stack chains accumulate in PSUM [128,256] half-bank tiles; DVE drains
them into per-gate z accumulators [128, 4*256] in SBUF (quad order
C11|C21|C12|C22 = gA b1|gA b2|gB b1|gB b2) with ACT doing the first-touch
copies and all gate activations. M slot order M4,M5,M7,M1,M2,M3,M6 completes
gateA after slot 4 and gateB halves after slots 5/6 so combines overlap the
next slots' matmuls. Phases 0-1 instead run direct quad products from raw
E,G,H,F quarters (smaller startup DMA, no combo dependencies; the a1 B/C
quarters stream twice through the PE). Outputs stream back as fp16.
"""

import sys

sys.path.insert(0, "/opt/trn_rl_repo")

import numpy as np

import concourse.bacc as bacc
import concourse.mybir as mybir
import concourse.tile as tile
from concourse.bass_utils import run_bass_kernel_spmd

N_CORES = 8
B, D, U = 4096, 1024, 1024
BS = B // N_CORES          # 512 batch rows per core
HB = BS // 2               # 256 batch half
P = 128
KH = 8                     # k-blocks per k-half (x-part / h-part)
NJ = U // P                # 8 u-blocks
NPH = 2 * NJ               # 16 phases
PAIRS = ((0, 2), (1, 3))   # (i, c~), (f, o)
F32 = mybir.dt.float32
F16 = mybir.dt.float16
ADD = mybir.AluOpType.add
SUB = mybir.AluOpType.subtract
MULT = mybir.AluOpType.mult
MIN = mybir.AluOpType.min

# M slots: (name, moving, stationary, [(quad_colbase, sign)...])
# moving: ("raw", a2slot, a3slot) uses araw slots + a1 tile; ("cmb", c) uses acm
# stationary: ("raw", kindbase) uses wraw; ("cmb",) uses the slot's jit tile
# wraw dim1: E:0-2 G:3-5 H:6-8 F:9-11 (x3 stacks); araw slots:
# a2A0 a3A1 a2D2 a3D3 a2B4 a3B5 a2C6 a3C7
# acm combos c: T1(A+D)=0 T2(C+D)=1 T5(A+B)=2 T6(C-A)=3 T7(B-D)=4
# quads in accum [P, 4*HB]: C11@0 C21@256 C12@512 C22@768
WB = {"GE": 0, "H": 3, "GH": 6, "EH": 9, "E": 12, "FH": 15, "EF": 18}
SLOTS = (
    ("M4", ("rawD",), "GE", ((0, 1), (256, 1))),
    ("M5", ("cmb", 2), "H", ((0, -1), (512, 1))),
    ("M7", ("cmb", 4), "GH", ((0, 1),)),
    ("M1", ("cmb", 0), "EH", ((0, 1), (768, 1))),
    ("M2", ("cmb", 1), "E", ((256, 1), (768, -1))),
    ("M3", ("rawA",), "FH", ((512, 1), (768, 1))),
    ("M6", ("cmb", 3), "EF", ((768, 1),)),
)

_CACHE = {}


def _build():
    nc = bacc.Bacc("TRN2", target_bir_lowering=False, debug=False,
                   num_devices=N_CORES)
    Tanh = mybir.ActivationFunctionType.Tanh
    Relu = mybir.ActivationFunctionType.Relu

    aq_in = nc.dram_tensor("aq", [P, 8 * KH * HB], F16, kind="ExternalInput").ap()
    wq_in = nc.dram_tensor("wq", [P, NPH * 21 * KH * P], F16,
                           kind="ExternalInput").ap()
    w01_in = nc.dram_tensor("wq01", [P, 2 * 12 * KH * P], F16,
                            kind="ExternalInput").ap()
    cprev_in = nc.dram_tensor("c_prevT", [P, 2 * NJ * BS], F32,
                              kind="ExternalInput").ap()
    brT = nc.dram_tensor("brT", [P, 4 * NJ], F32, kind="ExternalInput").ap()
    biT = nc.dram_tensor("biT", [P, 4 * NJ], F32, kind="ExternalInput").ap()
    h_outT = nc.dram_tensor("h_outT", [2 * U, BS], F16, kind="ExternalOutput").ap()
    c_outT = nc.dram_tensor("c_outT", [2 * U, BS], F16, kind="ExternalOutput").ap()

    WPH = 21 * KH * P   # weight elems per phase per partition

    with tile.TileContext(nc) as tc:
        with (
            tc.tile_pool(name="acts", bufs=1) as acts_p,
            tc.tile_pool(name="wraw", bufs=2) as wraw_p,
            tc.tile_pool(name="accum", bufs=2) as acc_p,
            tc.tile_pool(name="cprev", bufs=4) as cp_p,
            tc.tile_pool(name="bias", bufs=4) as bias_p,
            tc.tile_pool(name="gat", bufs=5) as gat_p,
            tc.tile_pool(name="out", bufs=5) as out_p,
            tc.tile_pool(name="psum", bufs=8, space="PSUM") as psum_p,
        ):
            # --- PE p-state warmup on zeros while startup DMA streams
            warm = out_p.tile([P, BS], F16, tag="out", name="warm")
            nc.gpsimd.memset(warm[:], 0)
            wps = psum_p.tile([P, BS], F32, tag="ps", name="warm_ps")
            for _ in range(12):
                nc.tensor.matmul(wps[:], warm[:, :P], warm[:],
                                 start=True, stop=True)

            # --- startup DMAs ------------------------------------------------
            # acts quarters, critical order A, D, B, C. A/D persist; B/C
            # share the cprev ring (dead after combo builds)
            aslot = []
            for q in range(8):
                if q < 4:
                    t = acts_p.tile([P, KH, HB], F16, tag="arawAD", bufs=4,
                                    name=f"araw{q}")
                else:
                    t = cp_p.tile([P, KH, HB], F16, tag="cp", bufs=4,
                                  name=f"araw{q}")
                aslot.append(t)
                eng = nc.scalar if q < 4 else nc.gpsimd
                eng.dma_start(t[:], aq_in[:, q * KH * HB:
                                          (q + 1) * KH * HB])

            wtiles = {}

            def wdma(ph):
                wt = wraw_p.tile([P, 21, KH, P], F16, tag="wraw",
                                 name=f"wraw{ph}")
                wtiles[ph] = wt
                for c0, c1 in ((0, 12), (12, 21)):
                    nc.sync.dma_start(
                        wt[:, c0:c1, :, :],
                        wq_in[:, ph * WPH + c0 * KH * P:
                              ph * WPH + c1 * KH * P])
                return wt

            # phases 0/1 stream raw E,G,H,F (smaller startup DMA); their
            # combos are not needed: those phases run direct quad products
            W01 = 12 * KH * P

            def wdma01(ph):
                wt = wraw_p.tile([P, 12, KH, P], F16, tag="wraw",
                                 name=f"wraw01_{ph}")
                wtiles[ph] = wt
                for c0, c1 in ((0, 6), (6, 12)):
                    nc.sync.dma_start(
                        wt[:, c0:c1, :, :],
                        w01_in[:, ph * W01 + c0 * KH * P:
                               ph * W01 + c1 * KH * P])

            wdma01(0)
            wdma01(1)

            # biases (small, needed at phase0 combine)
            braw, bhs = [], []
            for nm, din in (("brT", brT), ("biT", biT)):
                t = bias_p.tile([P, 4 * NJ], F32, tag="bias", name=f"braw_{nm}")
                nc.gpsimd.dma_start(t[:], din[:, :])
                braw.append(t)
                t2 = bias_p.tile([P, 4 * NJ], F32, tag="bias", name=f"bhs_{nm}")
                nc.vector.tensor_scalar(t2[:], t[:], 0.2, 0.5, MULT, ADD)
                bhs.append(t2)

            # --- act combos (one-time) --------------------------------------
            # a1 raw A/D quarters + 5 combos x 3 stacks; a1 combos derived
            a1A = acts_p.tile([P, KH, HB], F16, tag="a1A", name="a1A")
            a1D = acts_p.tile([P, KH, HB], F16, tag="a1D", name="a1D")
            acmt = [acts_p.tile([P, KH, HB], F16, tag="acm", bufs=15,
                                name=f"acm{i}") for i in range(15)]
            QA = {1: 0, 2: 1}   # stack -> araw slot of quarter A (a2A, a3A)
            QD = {1: 2, 2: 3}
            QB = {1: 4, 2: 5}
            QC = {1: 6, 2: 7}
            nc.vector.tensor_tensor(a1A[:], aslot[0][:], aslot[1][:], ADD)
            nc.vector.tensor_tensor(a1D[:], aslot[2][:], aslot[3][:], ADD)
            # a1 B/C quarters for the direct phases live in borrowed acm-ring
            # buffers (acmt[0],acmt[1]); the T1 combos overwrite them during
            # phase 1, after the last direct-phase read
            a1B = acts_p.tile([P, KH, HB], F16, tag="acm", bufs=15,
                              name="a1B")
            a1C = acts_p.tile([P, KH, HB], F16, tag="acm", bufs=15,
                              name="a1C")
            nc.vector.tensor_tensor(a1B[:], aslot[4][:], aslot[5][:], ADD)
            nc.vector.tensor_tensor(a1C[:], aslot[6][:], aslot[7][:], ADD)
            # combos: (c, q0map, q1map, op); built during phase 1
            CDEFS = ((2, QA, QB, ADD), (4, QB, QD, SUB), (1, QC, QD, ADD),
                     (3, QC, QA, SUB), (0, QA, QD, ADD))

            def acm_build():
                for c, q0, q1, op in CDEFS:
                    for s in (1, 2):
                        nc.vector.tensor_tensor(acmt[c * 3 + s][:],
                                                aslot[q0[s]][:],
                                                aslot[q1[s]][:], op)
                    nc.vector.tensor_tensor(acmt[c * 3][:], acmt[c * 3 + 1][:],
                                            acmt[c * 3 + 2][:], ADD)

            def moving(mv, s, k):
                if mv[0] == "rawD":
                    return a1D[:, k, :] if s == 0 else aslot[QD[s]][:, k, :]
                if mv[0] == "rawA":
                    return a1A[:, k, :] if s == 0 else aslot[QA[s]][:, k, :]
                return acmt[mv[1] * 3 + s][:, k, :]

            # state carried across phases
            t2s = {}
            cpt = {}

            RB = {"E": 0, "G": 3, "H": 6, "F": 9}
            DSLOTS = (
                (0, ("A", "B"), ("E", "G"), None),
                (256, ("C", "D"), ("E", "G"), "gA"),
                (512, ("A", "B"), ("F", "H"), "gB0"),
                (768, ("C", "D"), ("F", "H"), "gB1"),
            )
            DQ = {"A": (a1A, 0, 1), "D": (a1D, 2, 3),
                  "B": (a1B, 4, 5), "C": (a1C, 6, 7)}

            def dmov(qq, s):
                t0, i1, i2 = DQ[qq]
                if s == 1:
                    return [aslot[i1]]
                if s == 2:
                    return [aslot[i2]]
                return [t0] if t0 is not None else [aslot[i1], aslot[i2]]

            for ph in range(NPH):
                j, pair = divmod(ph, 2)
                gA, gB = PAIRS[pair]
                wt = wtiles.pop(ph)
                if pair == 1:
                    t = cp_p.tile([P, 2, BS], F32, tag="cp", bufs=4,
                                  name=f"cp{j}")
                    nc.sync.dma_start(t[:], cprev_in[:, 2 * j * BS:
                                                     (2 * j + 2) * BS])
                    cpt[j] = t
                if ph + 2 < NPH:
                    wdma(ph + 2)
                if ph == 1:
                    acm_build()

                acc = [acc_p.tile([P, 4 * HB], F32, tag="acc",
                                  name=f"acc_{ph}_{z}") for z in range(2)]
                state = {}

                def combine_gA(ph=ph, j=j, pair=pair, gA=gA, acc=acc,
                               state=state):
                    for z in range(2):
                        g = gat_p.tile([P, BS], F16, tag="gat",
                                       name=f"ga{ph}{z}")
                        nc.scalar.activation(
                            g[:], acc[z][:, :BS], Relu,
                            bias=bhs[z][:, gA * NJ + j:gA * NJ + j + 1],
                            scale=0.2)
                        state.setdefault("igs", []).append(g)

                def combine_fh(hf, ph=ph, j=j, gA=gA, acc=acc, state=state):
                    gsl = slice(hf * HB, (hf + 1) * HB)
                    if hf == 0:
                        state["fch"] = [
                            (gat_p.tile([P, BS], F16, tag="gat",
                                        name=f"fg{ph}{z}"),
                             gat_p.tile([P, BS], F16, tag="gat",
                                        name=f"t1{ph}{z}"),
                             gat_p.tile([P, BS], F16, tag="gat",
                                        name=f"tc2{ph}{z}"),
                             out_p.tile([P, BS], F16, tag="out",
                                        name=f"cn{ph}{z}"))
                            for z in range(2)]
                    for z in range(2):
                        fg, t1, tc2, cn = state["fch"][z]
                        nc.scalar.activation(
                            fg[:, gsl], acc[z][:, gsl], Relu,
                            bias=bhs[z][:, gA * NJ + j:gA * NJ + j + 1],
                            scale=0.2)
                        nc.vector.scalar_tensor_tensor(
                            t1[:, gsl], fg[:, gsl], 1.0,
                            cpt[j][:, z, gsl], MIN, MULT)
                        nc.vector.tensor_tensor(cn[:, gsl], t1[:, gsl],
                                                t2s[z][:, gsl], ADD)
                        rows0 = z * U + j * P
                        nc.sync.dma_start(c_outT[rows0:rows0 + P, gsl],
                                          cn[:, gsl])
                        nc.scalar.activation(tc2[:, gsl], cn[:, gsl], Tanh)
                        t2s[("tc2", z)] = tc2

                def combine_gB(hf, ph=ph, j=j, pair=pair, gB=gB, acc=acc,
                               state=state):
                    asl = slice(BS + hf * HB, BS + (hf + 1) * HB)
                    gsl = slice(hf * HB, (hf + 1) * HB)
                    if pair == 0:
                        if hf == 0:
                            state["gbt"] = [
                                (gat_p.tile([P, BS], F16, tag="gat",
                                            name=f"tt{ph}{z}"),
                                 gat_p.tile([P, BS], F16, tag="gat",
                                            name=f"t2{ph}{z}"))
                                for z in range(2)]
                        for z in range(2):
                            tt, t2 = state["gbt"][z]
                            nc.scalar.activation(
                                tt[:, gsl], acc[z][:, asl], Tanh,
                                bias=braw[z][:, gB * NJ + j:gB * NJ + j + 1],
                                scale=1.0)
                            nc.vector.scalar_tensor_tensor(
                                t2[:, gsl], state["igs"][z][:, gsl], 1.0,
                                tt[:, gsl], MIN, MULT)
                            t2s[z] = t2
                    else:
                        if hf == 0:
                            state["gbo"] = [
                                (gat_p.tile([P, BS], F16, tag="gat",
                                            name=f"og{ph}{z}"),
                                 out_p.tile([P, BS], F16, tag="out",
                                            name=f"hn{ph}{z}"))
                                for z in range(2)]
                        for z in range(2):
                            og, hn = state["gbo"][z]
                            nc.scalar.activation(
                                og[:, gsl], acc[z][:, asl], Relu,
                                bias=bhs[z][:, gB * NJ + j:gB * NJ + j + 1],
                                scale=0.2)
                            nc.vector.scalar_tensor_tensor(
                                hn[:, gsl], og[:, gsl], 1.0,
                                t2s[("tc2", z)][:, gsl], MIN, MULT)
                            rows0 = z * U + j * P
                            nc.scalar.dma_start(
                                h_outT[rows0:rows0 + P, gsl], hn[:, gsl])

                if ph < 2:
                    # direct quad products from raw E,G,H,F: no combo deps,
                    # smaller startup DMA; a1 B/C quarters stream twice on PE
                    for cb, qs, sts, trig in DSLOTS:
                        pa = psum_p.tile([P, BS], F32, tag="ps",
                                         name=f"dpa{ph}{cb}")
                        pb = psum_p.tile([P, BS], F32, tag="ps",
                                         name=f"dpb{ph}{cb}")
                        ps = (pa[:, :HB], pa[:, HB:], pb[:, :HB])
                        for s in (1, 2, 0):
                            seq = []
                            for qq, kind in zip(qs, sts):
                                for m in dmov(qq, s):
                                    seq.append((kind, m))
                            n = len(seq) * KH
                            t = 0
                            for kind, m in seq:
                                for k in range(KH):
                                    nc.tensor.matmul(
                                        ps[s], wt[:, RB[kind] + s, k, :],
                                        m[:, k, :],
                                        start=(t == 0), stop=(t == n - 1))
                                    t += 1
                        for z, sx in ((0, 2), (1, 1)):
                            a = acc[z][:, cb:cb + HB]
                            nc.scalar.copy(a, ps[0])
                            nc.vector.tensor_tensor(a, a, ps[sx], ADD)
                        if trig == "gA":
                            if pair == 0:
                                combine_gA()
                            else:
                                combine_fh(0)
                                combine_fh(1)
                        elif trig == "gB0":
                            combine_gB(0)
                        elif trig == "gB1":
                            combine_gB(1)
                    continue

                inited = set()
                for si, (mn, mv, st, uses) in enumerate(SLOTS):
                    pa = psum_p.tile([P, BS], F32, tag="ps", name=f"pa{ph}{si}")
                    pb = psum_p.tile([P, BS], F32, tag="ps", name=f"pb{ph}{si}")
                    ps = (pa[:, :HB], pa[:, HB:], pb[:, :HB])  # s0, s1, s2
                    for s in (1, 2, 0):
                        stat = wt[:, WB[st] + s]
                        for k in range(KH):
                            nc.tensor.matmul(ps[s], stat[:, k, :],
                                             moving(mv, s, k),
                                             start=(k == 0), stop=(k == KH - 1))
                    # drains: zr uses s0+s2, zi uses s0+s1; init via ACT copy
                    for cb, sgn in uses:
                        for z, sx in ((0, 2), (1, 1)):
                            a = acc[z][:, cb:cb + HB]
                            if (cb, z) not in inited:
                                inited.add((cb, z))
                                assert sgn == 1
                                nc.scalar.copy(a, ps[0])
                                nc.vector.tensor_tensor(a, a, ps[sx], ADD)
                            else:
                                op = ADD if sgn == 1 else SUB
                                nc.vector.tensor_tensor(a, a, ps[0], op)
                                nc.vector.tensor_tensor(a, a, ps[sx], op)

                    if si == 3 and pair == 1:
                        combine_fh(0)
                    elif si == 4:
                        combine_fh(1) if pair == 1 else combine_gA()
                    elif si in (5, 6):
                        combine_gB(si - 5)

    nc.compile()
    return nc


def _in_maps(inputs, h_tm1, c_tm1, wr, wi, wrr, wir, br, bi):
    brT = np.ascontiguousarray(br.reshape(4 * NJ, P).T)
    biT = np.ascontiguousarray(bi.reshape(4 * NJ, P).T)

    # Gauss weight stacks
    W1 = np.concatenate([wr, wrr], 0)
    W2 = np.concatenate([-(wi + wr), -(wir + wrr)], 0)
    W3 = np.concatenate([wi - wr, wir - wrr], 0)
    # 7 stationaries per phase/stack: raw E,H + 5 host-built combos
    WQ = np.empty((NPH, 21, P, KH, P), np.float16)
    for s, Ws in enumerate((W1, W2, W3)):
        v = Ws.reshape(2, KH, P, 4, NJ, P)
        for ph in range(NPH):
            j, pair = divmod(ph, 2)
            gA, gB = PAIRS[pair]
            E = v[0, :, :, gA, j, :]
            G = v[1, :, :, gA, j, :]
            H = v[1, :, :, gB, j, :]
            F = v[0, :, :, gB, j, :]
            for kind, arr in (("GE", G - E), ("H", H), ("GH", G + H),
                              ("EH", E + H), ("E", E), ("FH", F - H),
                              ("EF", E + F)):
                WQ[ph, WB[kind] + s] = arr.astype(np.float16).transpose(1, 0, 2)
    wq = np.ascontiguousarray(WQ.transpose(2, 0, 1, 3, 4).reshape(P, -1))
    RB = {"E": 0, "G": 3, "H": 6, "F": 9}
    W01 = np.empty((2, 12, P, KH, P), np.float16)
    for s, Ws in enumerate((W1, W2, W3)):
        v = Ws.reshape(2, KH, P, 4, NJ, P)
        for ph in range(2):
            gA, gB = PAIRS[ph]
            for kind, arr in (("E", v[0, :, :, gA, 0, :]),
                              ("G", v[1, :, :, gA, 0, :]),
                              ("H", v[1, :, :, gB, 0, :]),
                              ("F", v[0, :, :, gB, 0, :])):
                W01[ph, RB[kind] + s] = arr.astype(np.float16).transpose(1, 0, 2)
    wq01 = np.ascontiguousarray(W01.transpose(2, 0, 1, 3, 4).reshape(P, -1))

    maps = []
    for c in range(N_CORES):
        rows = slice(c * BS, (c + 1) * BS)
        xr, xi_ = inputs[rows, :D], inputs[rows, D:]
        hr, hi = h_tm1[rows, :U], h_tm1[rows, U:]
        a2 = np.empty((D + U, BS), np.float16)
        a2[:D] = xr.T
        a2[D:] = hr.T
        a3 = np.empty((D + U, BS), np.float16)
        a3[:D] = xi_.T
        a3[D:] = hi.T
        # quarter slots: a2A a3A a2D a3D a2B a3B a2C a3C
        # A=(k1,b1) B=(k2,b1) C=(k1,b2) D=(k2,b2)
        aq = np.empty((8, P, KH, HB), np.float16)
        for i, (src, kh, bh) in enumerate((
                (a2, 0, 0), (a3, 0, 0), (a2, 1, 1), (a3, 1, 1),
                (a2, 1, 0), (a3, 1, 0), (a2, 0, 1), (a3, 0, 1))):
            q = src[kh * 1024:(kh + 1) * 1024, bh * HB:(bh + 1) * HB]
            aq[i] = q.reshape(KH, P, HB).transpose(1, 0, 2)
        aqm = np.ascontiguousarray(aq.transpose(1, 0, 2, 3).reshape(P, -1))

        cpv = c_tm1[rows].T.reshape(2, NJ, P, BS).transpose(2, 1, 0, 3)
        cpv = np.ascontiguousarray(cpv.reshape(P, 2 * NJ * BS), np.float32)
        maps.append({"aq": aqm, "wq": wq, "wq01": wq01, "c_prevT": cpv,
                     "brT": brT, "biT": biT})
    return maps


def kernel(inputs, h_tm1, c_tm1, real_kernel, imaginary_kernel,
           real_recurrent_kernel, imaginary_recurrent_kernel,
           real_bias, imaginary_bias):
    if "nc" not in _CACHE:
        _CACHE["nc"] = _build()
    nc = _CACHE["nc"]

    maps = _in_maps(
        np.ascontiguousarray(inputs, dtype=np.float32),
        np.ascontiguousarray(h_tm1, dtype=np.float32),
        np.ascontiguousarray(c_tm1, dtype=np.float32),
        np.ascontiguousarray(real_kernel, dtype=np.float32),
        np.ascontiguousarray(imaginary_kernel, dtype=np.float32),
        np.ascontiguousarray(real_recurrent_kernel, dtype=np.float32),
        np.ascontiguousarray(imaginary_recurrent_kernel, dtype=np.float32),
        np.ascontiguousarray(real_bias, dtype=np.float32),
        np.ascontiguousarray(imaginary_bias, dtype=np.float32),
    )
    res = run_bass_kernel_spmd(nc, maps, list(range(N_CORES)))
    h = np.concatenate(
        [res.results[c]["h_outT"].T.astype(np.float32)
         for c in range(N_CORES)], axis=0)
    c = np.concatenate(
        [res.results[c]["c_outT"].T.astype(np.float32)
         for c in range(N_CORES)], axis=0)
    return np.ascontiguousarray(h), np.ascontiguousarray(c)
